# revision 47
# baseline (speedup 1.0000x reference)
"""Distributed Trainium2 kernel for causal multi-head attention with RoPE.

Problem: hidden[2,2048,512] -> qkv proj (8 heads x 64) -> RoPE -> causal
attention -> out proj [512,512] -> out [2,2048,512].

Sharding: 8 cores = (2 batches) x (4 head-pairs). Each core computes the
full attention pipeline for its batch and its 2 heads plus its slice of
the output projection (contraction over its 128 w_o rows); the host sums
the 4 partial outputs per batch.

v2 design notes (vs the v1 baseline at 157us):
- hidden pre-transposed + bf16 on host: no on-device transposes, half DMA.
- RoPE rotate-half via SBUF->SBUF DMA partition shift with a sign-folded
  sin table (s2n = -rotate_half-layout sin), so no extra rot matmul cols.
- scores: 2 heads row-packed in the PE array (K=64 each at tile rows 0/64)
  -> concurrent matmuls, 2x score throughput.
- exp on ScalarE in [128,1024] PSUM chunks (2 score blocks per ACTIVATE)
  to amortize the ~220cyc instruction overhead.
- causal mask multiply only on the 128-wide triangle slice of diagonal
  blocks (the rest of a diagonal block is unmasked).
- V in [k,d] layout via DMA-transpose; shared ones-columns give the
  softmax denominators as row 64 of the AV accumulators.
- normalization: l rows -> f32 stage tiles -> reciprocal_approx_fast ->
  ones-outer-product broadcast matmuls -> aligned tensor-tensor multiply.
- single summed [2048,512] bf16 output per core; host sums 4 per batch.
"""

import sys

import numpy as np

sys.path.insert(0, "/opt/trn_rl_repo")

import ml_dtypes  # noqa: E402

import concourse.bass as bass  # noqa: E402
import concourse.mybir as mybir  # noqa: E402
import concourse.tile as tile  # noqa: E402
from concourse import bacc  # noqa: E402
from concourse.bass_utils import run_bass_kernel_spmd  # noqa: E402

B, S, HID = 2, 2048, 512
F32 = mybir.dt.float32
BF16 = mybir.dt.bfloat16
BF16NP = ml_dtypes.bfloat16

_CACHE = {}
_DEBUG = False


def _build():
    nc = bacc.Bacc(None)

    hidT = nc.declare_dram_parameter("hidT", [HID, S], BF16, isOutput=False)
    wc = nc.declare_dram_parameter("wc", [HID, 384], BF16, isOutput=False)
    c2d = nc.declare_dram_parameter("c2d", [128, S], BF16, isOutput=False)
    s2d = nc.declare_dram_parameter("s2d", [128, S], BF16, isOutput=False)
    trid = nc.declare_dram_parameter("trid", [128, 128], BF16, isOutput=False)
    identd = nc.declare_dram_parameter("identd", [128, 128], BF16,
                                       isOutput=False)
    wod = nc.declare_dram_parameter("wod", [128, HID], BF16, isOutput=False)
    out = nc.declare_dram_parameter("out", [S, HID], BF16, isOutput=True)

    Exp = mybir.ActivationFunctionType.Exp

    with tile.TileContext(nc) as tc, \
         tc.tile_pool(name="const", bufs=1) as constp, \
         tc.tile_pool(name="big", bufs=1) as bigp, \
         tc.tile_pool(name="work", bufs=4) as workp, \
         tc.tile_pool(name="ps", bufs=2, space="PSUM") as psp:

        # ---- constants / inputs: hidden on the sync queue, weights and
        # tables on the scalar queue, so the first QKV matmul's operands
        # land as early as possible ----
        hT = []
        for kc in range(4):
            t = bigp.tile([128, S], BF16, name=f"hT{kc}")
            eng = nc.sync if kc % 2 == 0 else nc.gpsimd
            eng.dma_start(t[:], hidT[kc * 128:(kc + 1) * 128, :])
            hT.append(t)
        wcs = constp.tile([128, 4 * 384], BF16, name="wcs")
        for kc in range(4):
            nc.scalar.dma_start(wcs[:, kc * 384:(kc + 1) * 384],
                                wc[kc * 128:(kc + 1) * 128, :])
        c2 = constp.tile([128, S], BF16, name="c2")
        nc.scalar.dma_start(c2[:], c2d[:])
        s2 = constp.tile([128, S], BF16, name="s2")
        nc.scalar.dma_start(s2[:], s2d[:])
        tri = constp.tile([128, 128], BF16, name="tri")
        nc.scalar.dma_start(tri[:], trid[:])
        identb = constp.tile([128, 128], BF16, name="identb")
        nc.scalar.dma_start(identb[:], identd[:])
        wob = constp.tile([128, HID], BF16, name="wob")
        nc.scalar.dma_start(wob[:], wod[:])
        ones1 = constp.tile([1, 64], BF16, name="ones1")
        nc.vector.memset(ones1[:], 1.0)

        # persistent activations
        qt = bigp.tile([128, S], BF16, name="qt")
        kt = bigp.tile([128, S], BF16, name="kt")
        vT = bigp.tile([128, S], BF16, name="vT")
        stageA = bigp.tile([65, S], F32, name="stageA")
        stageB = bigp.tile([65, S], F32, name="stageB")
        attnT = bigp.tile([128, S], BF16, name="attnT")
        attnB = bigp.tile([64, S], BF16, name="attnB")
        l2fA = bigp.tile([1, S], F32, name="l2fA")
        l2fB = bigp.tile([1, S], F32, name="l2fB")
        recAb = bigp.tile([1, S], BF16, name="recAb")
        recBb = bigp.tile([1, S], BF16, name="recBb")

        # v in [k, d] layout, 16 blocks of [128, 131]:
        # cols 0:64 = v_h0, 64 = ones, 65 unused, 66:130 = v_h1, 130 = ones
        vx = []
        for kb in range(16):
            t = bigp.tile([128, 131], BF16, name=f"vx{kb}")
            vx.append(t)

        def qkv_pieces(n):
            # wc col groups: [q2 | k2 | v2], 128 each. rotate_half comes
            # from an SBUF->SBUF DMA partition rotate-by-32 of u1 = q*s2n
            # (s2n is the sign-folded shifted sin table). Returns emission
            # closures so attention can interleave them into its exp-wait
            # gaps (keeps the PE warm, ACT never idles).
            nsl = slice(n * 512, (n + 1) * 512)
            pieces = []

            def rope(dst, lo):
                ps = psp.tile([128, 512], F32, name=f"ps{n}_{lo}",
                              tag="qkv", bufs=2)
                for kc in range(4):
                    nc.tensor.matmul(
                        ps[:],
                        wcs[:, kc * 384 + lo:kc * 384 + lo + 128],
                        hT[kc][:, nsl],
                        start=(kc == 0), stop=(kc == 3),
                    )
                t1 = workp.tile([128, 512], BF16, name="t1", tag="t1", bufs=2)
                nc.vector.tensor_mul(t1[:], ps[:], c2[:, nsl])
                u1 = workp.tile([128, 512], BF16, name="u1", tag="u1", bufs=2)
                nc.vector.tensor_mul(u1[:], ps[:], s2[:, nsl])
                u2 = workp.tile([128, 512], BF16, name="u2", tag="u2", bufs=2)
                for h in range(2):
                    o = h * 64
                    nc.sync.dma_start(u2[o:o + 32, :], u1[o + 32:o + 64, :])
                    nc.sync.dma_start(u2[o + 32:o + 64, :], u1[o:o + 32, :])
                nc.vector.tensor_add(dst[:, nsl], t1[:], u2[:])

            def vproj():
                ps = psp.tile([128, 512], F32, name=f"psv{n}",
                              tag="qkv", bufs=2)
                for kc in range(4):
                    nc.tensor.matmul(
                        ps[:],
                        wcs[:, kc * 384 + 256:kc * 384 + 384],
                        hT[kc][:, nsl],
                        start=(kc == 0), stop=(kc == 3),
                    )
                nc.vector.tensor_copy(vT[:, nsl], ps[:])

            def vxa():
                # transpose v chunk on the PE, then split columns on gpsimd
                tpv = psp.tile([128, 512], BF16, name=f"tpv{n}", tag="qkv",
                               bufs=2)
                for i in range(4):
                    kb = 4 * n + i
                    nc.tensor.transpose(tpv[:, i * 128:(i + 1) * 128],
                                        vT[:, kb * 128:(kb + 1) * 128],
                                        identb[:])
                vt4 = workp.tile([128, 512], BF16, name="vt4", tag="vt4",
                                 bufs=2)
                nc.vector.tensor_copy(vt4[:], tpv[:])
                for i in range(4):
                    kb = 4 * n + i
                    nc.gpsimd.tensor_copy(vx[kb][:, 0:64],
                                          vt4[:, i * 128:i * 128 + 64])
                    nc.gpsimd.tensor_copy(vx[kb][:, 66:130],
                                          vt4[:, i * 128 + 64:i * 128 + 128])
                    nc.vector.memset(vx[kb][:, 64:65], 1.0)
                    nc.vector.memset(vx[kb][:, 130:131], 1.0)

            pieces.append(lambda: rope(qt, 0))
            pieces.append(lambda: rope(kt, 128))
            pieces.append(vproj)
            pieces.append(vxa)
            return pieces

        def attn_qc(qc, fillers=(), prelude=()):
            fillers = list(fillers)
            nkb = 4 * qc + 4
            qbase = qc * 512
            accs = []
            for h in range(2):
                acc = psp.tile([128, 512], F32, name=f"acc{h}_{qc}",
                               tag="acc", bufs=2)
                accs.append(acc)
            # chunks of 2 kb blocks; diag block j (= kb - 4*qc >= 0) is
            # trimmed to live q cols [128j, 512)
            chunks = []
            for kb0 in range(0, nkb, 2):
                blocks = []  # (kb, q0b, psum col offset, len)
                off = 0
                for kb in range(kb0, min(kb0 + 2, nkb)):
                    j = kb - 4 * qc
                    q0b = max(0, 128 * j)
                    ln = 512 - q0b
                    blocks.append((kb, q0b, off, ln))
                    off += ln
                chunks.append((blocks, off))

            scs = {}

            def mm_chunk(i, h):
                blocks, off = chunks[i]
                hsl = slice(h * 64, (h + 1) * 64)
                sc = psp.tile([128, 1024], F32, name=f"sc{h}",
                              tag="sc", bufs=2)
                for kb, q0b, o, ln in blocks:
                    nc.tensor.matmul(
                        sc[:, o:o + ln],
                        kt[hsl, kb * 128:(kb + 1) * 128],
                        qt[hsl, qbase + q0b:qbase + 512],
                        start=True, stop=True,
                    )
                scs[(i, h)] = sc

            # software pipeline: exp/AV of chunk i interleave with the
            # score matmuls of chunk i+1 (per head, matching the sc ring)
            mm_chunk(0, 0)
            mm_chunk(0, 1)
            for p in prelude:
                p()  # emitted before any AV: AV inputs (vx) may come here
            for i, (blocks, off) in enumerate(chunks):
                for h in range(2):
                    pr = workp.tile([128, 1024], BF16, name=f"pr{h}",
                                    tag="probs", bufs=6)
                    nc.scalar.activation(pr[:, 0:off], scs.pop((i, h))[:, 0:off],
                                         Exp, scale=0.125)
                    lsl = slice(h * 65 + h, h * 65 + h + 65)  # 0:65 / 66:131
                    for kb, q0b, o, ln in blocks:
                        j = kb - 4 * qc
                        if j >= 0:
                            # mask the 128-wide triangle slice in place;
                            # the rest of a diagonal block is unmasked
                            nc.vector.tensor_mul(pr[:, o:o + 128],
                                                 pr[:, o:o + 128],
                                                 tri[:])
                        nc.tensor.matmul(
                            accs[h][0:65, q0b:512],
                            vx[kb][:, lsl],
                            pr[:, o:o + ln],
                            start=(kb == 0), stop=(kb == nkb - 1),
                        )
                    if i + 1 < len(chunks):
                        mm_chunk(i + 1, h)
                    if fillers:
                        fillers.pop(0)()
            for f in fillers:
                f()
            qsl = slice(qbase, qbase + 512)
            nc.vector.tensor_copy(stageA[:, qsl], accs[0][0:65, :])
            nc.vector.tensor_copy(stageB[:, qsl], accs[1][0:65, :])
            # denominators to base-0 f32 rows (partition shift via DMA)
            nc.sync.dma_start(l2fA[0:1, qsl], stageA[64:65, qsl])
            nc.sync.dma_start(l2fB[0:1, qsl], stageB[64:65, qsl])

        def tail_pieces(qc):
            # recip + normalize + outproj for one qc, as filler closures
            # emitted inside the NEXT qc's attention loop
            qsl = slice(qc * 512, (qc + 1) * 512)

            def recip():
                for lf, rb in ((l2fA, recAb), (l2fB, recBb)):
                    rec = workp.tile([1, 512], F32, name="rec", tag="rec",
                                     bufs=2)
                    nc.vector.reciprocal_approx_fast(rec[:], lf[0:1, qsl])
                    nc.vector.tensor_copy(rb[0:1, qsl], rec[:])

            def norm(h):
                recb = psp.tile([64, 512], F32, name=f"recb{h}",
                                tag="qkv", bufs=2)
                src = recAb if h == 0 else recBb
                nc.tensor.matmul(recb[:], ones1[:], src[0:1, qsl],
                                 start=True, stop=True)
                if h == 0:
                    nc.vector.tensor_mul(attnT[0:64, qsl],
                                         stageA[0:64, qsl], recb[:])
                else:
                    nc.vector.tensor_mul(attnB[0:64, qsl],
                                         stageB[0:64, qsl], recb[:])
                    nc.sync.dma_start(attnT[64:128, qsl], attnB[0:64, qsl])

            def outproj(mc):
                oP = psp.tile([128, 512], F32, name=f"oP{mc}",
                              tag="qkv", bufs=2)
                nc.tensor.matmul(oP[:], attnT[:, mc * 128:(mc + 1) * 128],
                                 wob[:], start=True, stop=True)
                osb = workp.tile([128, 512], BF16, name="osb", tag="osb",
                                 bufs=3)
                nc.vector.tensor_copy(osb[:], oP[:])
                eng = nc.gpsimd if mc % 2 == 0 else nc.sync
                eng.dma_start(out[mc * 128:(mc + 1) * 128, :], osb[:])

            pieces = [recip, lambda: norm(0), lambda: norm(1)]
            pieces.extend(
                (lambda m: lambda: outproj(m))(mc)
                for mc in range(4 * qc, 4 * qc + 4))
            return pieces

        # ---- pipelined main loop: qkv/vx for chunk n+1 AND the previous
        # qc's normalization/outproj tail are emitted as fillers inside
        # attention qc=n so the PE's exp-wait gaps get matmul work and the
        # scalar engine's exp stream never starves ----
        for piece in qkv_pieces(0):
            piece()
        prev_tail = []
        for n in range(4):
            fillers = (qkv_pieces(n + 1) if n + 1 < 4 else []) + prev_tail
            attn_qc(n, fillers)
            prev_tail = tail_pieces(n)
        for piece in prev_tail:
            piece()

        if _DEBUG:
            dbg_specs = [
                ("d_qt", qt, [128, S]),
                ("d_kt", kt, [128, S]),
                ("d_vT", vT, [128, S]),
                ("d_vx0", vx[0], [128, 131]),
                ("d_vx7", vx[7], [128, 131]),
                ("d_stageA", stageA, [65, S]),
                ("d_stageB", stageB, [65, S]),
                ("d_l2f", l2fA, [1, S]),
                ("d_l2rb", recAb, [1, S]),
                ("d_recBb", recBb, [1, S]),
                ("d_attnT", attnT, [128, S]),
            ]
            for nm, t, shp in dbg_specs:
                dt = F32 if t in (stageA, stageB, l2fA) else BF16
                d = nc.declare_dram_parameter(nm, shp, dt, isOutput=True)
                nc.sync.dma_start(d[:], t[:])

    nc.finalize()
    return nc


def _get_nc():
    if "nc" not in _CACHE:
        _CACHE["nc"] = _build()
    return _CACHE["nc"]


def kernel(hidden_states, cos, sin, w_qkv, w_o, _trace=False):
    hidden_states = np.asarray(hidden_states, dtype=np.float32)
    cos = np.asarray(cos, dtype=np.float32)
    sin = np.asarray(sin, dtype=np.float32)
    w_qkv = np.asarray(w_qkv, dtype=np.float32)
    w_o = np.asarray(w_o, dtype=np.float32)

    nc = _get_nc()

    # cos/sin tables in [d-row, position] layout, duplicated for 2 heads.
    # s2n is the sign-folded shifted sin: after the on-device partition
    # rotate-by-32 (u2[r] = u1[swap(r)]), u2 equals rotate_half(q) * sin.
    ct = cos.T  # [64, S]
    st = sin.T
    s2n_h = np.concatenate([st[32:64], -st[0:32]], axis=0)  # [64, S]
    c2 = np.concatenate([ct, ct], axis=0).astype(BF16NP)
    s2 = np.concatenate([s2n_h, s2n_h], axis=0).astype(BF16NP)

    kl = np.arange(128)[:, None]
    ql = np.arange(128)[None, :]
    tri = (kl <= ql).astype(BF16NP)
    ident = np.eye(128).astype(BF16NP)

    in_maps = []
    for c in range(8):
        b, g = c // 4, c % 4
        heads = (2 * g, 2 * g + 1)
        wq = [w_qkv[:, h * 64:(h + 1) * 64] for h in heads]
        wk = [w_qkv[:, 512 + h * 64:512 + (h + 1) * 64] for h in heads]
        wv = [w_qkv[:, 1024 + h * 64:1024 + (h + 1) * 64] for h in heads]
        wc = np.concatenate(
            [wq[0], wq[1], wk[0], wk[1], wv[0], wv[1]], axis=1)
        in_maps.append({
            "hidT": np.ascontiguousarray(
                hidden_states[b].T).astype(BF16NP),
            "wc": np.ascontiguousarray(wc).astype(BF16NP),
            "c2d": c2,
            "s2d": s2,
            "trid": tri,
            "identd": ident,
            "wod": np.ascontiguousarray(
                w_o[g * 128:(g + 1) * 128, :]).astype(BF16NP),
        })

    res = run_bass_kernel_spmd(nc, in_maps, list(range(8)), trace=_trace)
    _CACHE["last_result"] = res
    parts = [np.asarray(res.results[c]["out"], dtype=np.float32)
             for c in range(8)]
    full = np.stack([
        parts[0] + parts[1] + parts[2] + parts[3],
        parts[4] + parts[5] + parts[6] + parts[7],
    ])
    return full.astype(np.float32)


# revision 48
# speedup vs baseline: 1.0084x; 1.0084x over previous
"""Distributed Trainium2 kernel for causal multi-head attention with RoPE.

Problem: hidden[2,2048,512] -> qkv proj (8 heads x 64) -> RoPE -> causal
attention -> out proj [512,512] -> out [2,2048,512].

Sharding: 8 cores = (2 batches) x (4 head-pairs). Each core computes the
full attention pipeline for its batch and its 2 heads plus its slice of
the output projection (contraction over its 128 w_o rows); the host sums
the 4 partial outputs per batch.

v2 design notes (vs the v1 baseline at 157us):
- hidden pre-transposed + bf16 on host: no on-device transposes, half DMA.
- RoPE rotate-half via SBUF->SBUF DMA partition shift with a sign-folded
  sin table (s2n = -rotate_half-layout sin), so no extra rot matmul cols.
- scores: 2 heads row-packed in the PE array (K=64 each at tile rows 0/64)
  -> concurrent matmuls, 2x score throughput.
- exp on ScalarE in [128,1024] PSUM chunks (2 score blocks per ACTIVATE)
  to amortize the ~220cyc instruction overhead.
- causal mask multiply only on the 128-wide triangle slice of diagonal
  blocks (the rest of a diagonal block is unmasked).
- V in [k,d] layout via DMA-transpose; shared ones-columns give the
  softmax denominators as row 64 of the AV accumulators.
- normalization: l rows -> f32 stage tiles -> reciprocal_approx_fast ->
  ones-outer-product broadcast matmuls -> aligned tensor-tensor multiply.
- single summed [2048,512] bf16 output per core; host sums 4 per batch.
"""

import sys

import numpy as np

sys.path.insert(0, "/opt/trn_rl_repo")

import ml_dtypes  # noqa: E402

import concourse.bass as bass  # noqa: E402
import concourse.mybir as mybir  # noqa: E402
import concourse.tile as tile  # noqa: E402
from concourse import bacc  # noqa: E402
from concourse.bass_utils import run_bass_kernel_spmd  # noqa: E402

B, S, HID = 2, 2048, 512
F32 = mybir.dt.float32
BF16 = mybir.dt.bfloat16
BF16NP = ml_dtypes.bfloat16

_CACHE = {}
_DEBUG = False


def _build():
    nc = bacc.Bacc(None)

    hidT = nc.declare_dram_parameter("hidT", [HID, S], BF16, isOutput=False)
    wc = nc.declare_dram_parameter("wc", [HID, 640], BF16, isOutput=False)
    c2d = nc.declare_dram_parameter("c2d", [128, S], BF16, isOutput=False)
    s2d = nc.declare_dram_parameter("s2d", [128, S], BF16, isOutput=False)
    trid = nc.declare_dram_parameter("trid", [128, 128], BF16, isOutput=False)
    identd = nc.declare_dram_parameter("identd", [128, 128], BF16,
                                       isOutput=False)
    wod = nc.declare_dram_parameter("wod", [128, HID], BF16, isOutput=False)
    out = nc.declare_dram_parameter("out", [S, HID], BF16, isOutput=True)

    Exp = mybir.ActivationFunctionType.Exp

    with tile.TileContext(nc) as tc, \
         tc.tile_pool(name="const", bufs=1) as constp, \
         tc.tile_pool(name="big", bufs=1) as bigp, \
         tc.tile_pool(name="work", bufs=4) as workp, \
         tc.tile_pool(name="ps", bufs=2, space="PSUM") as psp:

        # ---- constants / inputs: hidden on the sync queue, weights and
        # tables on the scalar queue, so the first QKV matmul's operands
        # land as early as possible ----
        hT = []
        for kc in range(4):
            t = bigp.tile([128, S], BF16, name=f"hT{kc}")
            eng = nc.sync if kc % 2 == 0 else nc.gpsimd
            eng.dma_start(t[:], hidT[kc * 128:(kc + 1) * 128, :])
            hT.append(t)
        wcs = constp.tile([128, 4 * 640], BF16, name="wcs")
        for kc in range(4):
            nc.scalar.dma_start(wcs[:, kc * 640:(kc + 1) * 640],
                                wc[kc * 128:(kc + 1) * 128, :])
        c2 = constp.tile([128, S], BF16, name="c2")
        nc.scalar.dma_start(c2[:], c2d[:])
        s2 = constp.tile([128, S], BF16, name="s2")
        nc.scalar.dma_start(s2[:], s2d[:])
        tri = constp.tile([128, 128], BF16, name="tri")
        nc.scalar.dma_start(tri[:], trid[:])
        identb = constp.tile([128, 128], BF16, name="identb")
        nc.scalar.dma_start(identb[:], identd[:])
        wob = constp.tile([128, HID], BF16, name="wob")
        nc.scalar.dma_start(wob[:], wod[:])
        ones1 = constp.tile([1, 64], BF16, name="ones1")
        nc.vector.memset(ones1[:], 1.0)

        # persistent activations
        qt = bigp.tile([128, S], BF16, name="qt")
        kt = bigp.tile([128, S], BF16, name="kt")
        vT = bigp.tile([128, S], BF16, name="vT")
        stageA = bigp.tile([65, S], F32, name="stageA")
        stageB = bigp.tile([65, S], F32, name="stageB")
        attnT = bigp.tile([128, S], BF16, name="attnT")
        attnB = bigp.tile([64, S], BF16, name="attnB")
        l2fA = bigp.tile([1, S], F32, name="l2fA")
        l2fB = bigp.tile([1, S], F32, name="l2fB")
        recAb = bigp.tile([1, S], BF16, name="recAb")
        recBb = bigp.tile([1, S], BF16, name="recBb")

        # v in [k, d] layout, 16 blocks of [128, 131]:
        # cols 0:64 = v_h0, 64 = ones, 65 unused, 66:130 = v_h1, 130 = ones
        vx = []
        for kb in range(16):
            t = bigp.tile([128, 131], BF16, name=f"vx{kb}")
            vx.append(t)

        def qkv_pieces(n):
            # wc col groups: [q2 | k2 | v2], 128 each. rotate_half comes
            # from an SBUF->SBUF DMA partition rotate-by-32 of u1 = q*s2n
            # (s2n is the sign-folded shifted sin table). Returns emission
            # closures so attention can interleave them into its exp-wait
            # gaps (keeps the PE warm, ACT never idles).
            nsl = slice(n * 512, (n + 1) * 512)
            pieces = []

            def rope(dst, lo):
                ps = psp.tile([128, 512], F32, name=f"ps{n}_{lo}",
                              tag="qkv", bufs=2)
                for kc in range(4):
                    nc.tensor.matmul(
                        ps[:],
                        wcs[:, kc * 640 + lo:kc * 640 + lo + 128],
                        hT[kc][:, nsl],
                        start=(kc == 0), stop=(kc == 3),
                    )
                psr = psp.tile([128, 512], F32, name=f"psr{n}_{lo}",
                               tag="qkv", bufs=2)
                for kc in range(4):
                    nc.tensor.matmul(
                        psr[:],
                        wcs[:, kc * 640 + lo + 128:kc * 640 + lo + 256],
                        hT[kc][:, nsl],
                        start=(kc == 0), stop=(kc == 3),
                    )
                t1 = workp.tile([128, 512], BF16, name="t1", tag="t1", bufs=2)
                nc.vector.tensor_mul(t1[:], ps[:], c2[:, nsl])
                t2 = workp.tile([128, 512], BF16, name="t2", tag="t2", bufs=2)
                nc.vector.tensor_mul(t2[:], psr[:], s2[:, nsl])
                nc.vector.tensor_add(dst[:, nsl], t1[:], t2[:])

            def vproj():
                ps = psp.tile([128, 512], F32, name=f"psv{n}",
                              tag="qkv", bufs=2)
                for kc in range(4):
                    nc.tensor.matmul(
                        ps[:],
                        wcs[:, kc * 640 + 512:kc * 640 + 640],
                        hT[kc][:, nsl],
                        start=(kc == 0), stop=(kc == 3),
                    )
                nc.vector.tensor_copy(vT[:, nsl], ps[:])

            def vxa():
                # transpose v chunk on the PE, then split columns on gpsimd
                tpv = psp.tile([128, 512], BF16, name=f"tpv{n}", tag="qkv",
                               bufs=2)
                for i in range(4):
                    kb = 4 * n + i
                    nc.tensor.transpose(tpv[:, i * 128:(i + 1) * 128],
                                        vT[:, kb * 128:(kb + 1) * 128],
                                        identb[:])
                vt4 = workp.tile([128, 512], BF16, name="vt4", tag="vt4",
                                 bufs=2)
                nc.vector.tensor_copy(vt4[:], tpv[:])
                for i in range(4):
                    kb = 4 * n + i
                    nc.gpsimd.tensor_copy(vx[kb][:, 0:64],
                                          vt4[:, i * 128:i * 128 + 64])
                    nc.gpsimd.tensor_copy(vx[kb][:, 66:130],
                                          vt4[:, i * 128 + 64:i * 128 + 128])
                    nc.vector.memset(vx[kb][:, 64:65], 1.0)
                    nc.vector.memset(vx[kb][:, 130:131], 1.0)

            pieces.append(lambda: rope(qt, 0))
            pieces.append(lambda: rope(kt, 256))
            pieces.append(vproj)
            pieces.append(vxa)
            return pieces

        def attn_qc(qc, fillers=(), prelude=()):
            fillers = list(fillers)
            nkb = 4 * qc + 4
            qbase = qc * 512
            accs = []
            for h in range(2):
                acc = psp.tile([128, 512], F32, name=f"acc{h}_{qc}",
                               tag="acc", bufs=2)
                accs.append(acc)
            # chunks of 2 kb blocks; diag block j (= kb - 4*qc >= 0) is
            # trimmed to live q cols [128j, 512)
            chunks = []
            for kb0 in range(0, nkb, 2):
                blocks = []  # (kb, q0b, psum col offset, len)
                off = 0
                for kb in range(kb0, min(kb0 + 2, nkb)):
                    j = kb - 4 * qc
                    q0b = max(0, 128 * j)
                    ln = 512 - q0b
                    blocks.append((kb, q0b, off, ln))
                    off += ln
                chunks.append((blocks, off))

            scs = {}

            def mm_chunk(i, h):
                blocks, off = chunks[i]
                hsl = slice(h * 64, (h + 1) * 64)
                sc = psp.tile([128, 1024], F32, name=f"sc{h}",
                              tag="sc", bufs=2)
                for kb, q0b, o, ln in blocks:
                    nc.tensor.matmul(
                        sc[:, o:o + ln],
                        kt[hsl, kb * 128:(kb + 1) * 128],
                        qt[hsl, qbase + q0b:qbase + 512],
                        start=True, stop=True,
                    )
                scs[(i, h)] = sc

            # software pipeline: exp/AV of chunk i interleave with the
            # score matmuls of chunk i+1 (per head, matching the sc ring)
            mm_chunk(0, 0)
            mm_chunk(0, 1)
            for p in prelude:
                p()  # emitted before any AV: AV inputs (vx) may come here
            for i, (blocks, off) in enumerate(chunks):
                for h in range(2):
                    pr = workp.tile([128, 1024], BF16, name=f"pr{h}",
                                    tag="probs", bufs=6)
                    nc.scalar.activation(pr[:, 0:off], scs.pop((i, h))[:, 0:off],
                                         Exp, scale=0.125)
                    lsl = slice(h * 65 + h, h * 65 + h + 65)  # 0:65 / 66:131
                    for kb, q0b, o, ln in blocks:
                        j = kb - 4 * qc
                        if j >= 0:
                            # mask the 128-wide triangle slice in place;
                            # the rest of a diagonal block is unmasked
                            nc.vector.tensor_mul(pr[:, o:o + 128],
                                                 pr[:, o:o + 128],
                                                 tri[:])
                        nc.tensor.matmul(
                            accs[h][0:65, q0b:512],
                            vx[kb][:, lsl],
                            pr[:, o:o + ln],
                            start=(kb == 0), stop=(kb == nkb - 1),
                        )
                    if i + 1 < len(chunks):
                        mm_chunk(i + 1, h)
                    if fillers:
                        fillers.pop(0)()
            for f in fillers:
                f()
            qsl = slice(qbase, qbase + 512)
            nc.vector.tensor_copy(stageA[:, qsl], accs[0][0:65, :])
            nc.vector.tensor_copy(stageB[:, qsl], accs[1][0:65, :])
            # denominators to base-0 f32 rows (partition shift via DMA)
            nc.sync.dma_start(l2fA[0:1, qsl], stageA[64:65, qsl])
            nc.sync.dma_start(l2fB[0:1, qsl], stageB[64:65, qsl])

        def tail_pieces(qc):
            # recip + normalize + outproj for one qc, as filler closures
            # emitted inside the NEXT qc's attention loop
            qsl = slice(qc * 512, (qc + 1) * 512)

            def recip():
                for lf, rb in ((l2fA, recAb), (l2fB, recBb)):
                    rec = workp.tile([1, 512], F32, name="rec", tag="rec",
                                     bufs=2)
                    nc.vector.reciprocal_approx_fast(rec[:], lf[0:1, qsl])
                    nc.vector.tensor_copy(rb[0:1, qsl], rec[:])

            def norm(h):
                recb = psp.tile([64, 512], F32, name=f"recb{h}",
                                tag="qkv", bufs=2)
                src = recAb if h == 0 else recBb
                nc.tensor.matmul(recb[:], ones1[:], src[0:1, qsl],
                                 start=True, stop=True)
                if h == 0:
                    nc.vector.tensor_mul(attnT[0:64, qsl],
                                         stageA[0:64, qsl], recb[:])
                else:
                    nc.vector.tensor_mul(attnB[0:64, qsl],
                                         stageB[0:64, qsl], recb[:])
                    nc.sync.dma_start(attnT[64:128, qsl], attnB[0:64, qsl])

            def outproj(mc):
                oP = psp.tile([128, 512], F32, name=f"oP{mc}",
                              tag="qkv", bufs=2)
                nc.tensor.matmul(oP[:], attnT[:, mc * 128:(mc + 1) * 128],
                                 wob[:], start=True, stop=True)
                osb = workp.tile([128, 512], BF16, name="osb", tag="osb",
                                 bufs=3)
                nc.vector.tensor_copy(osb[:], oP[:])
                eng = nc.gpsimd if mc % 2 == 0 else nc.sync
                eng.dma_start(out[mc * 128:(mc + 1) * 128, :], osb[:])

            pieces = [recip, lambda: norm(0), lambda: norm(1)]
            pieces.extend(
                (lambda m: lambda: outproj(m))(mc)
                for mc in range(4 * qc, 4 * qc + 4))
            return pieces

        # ---- pipelined main loop: qkv/vx for chunk n+1 AND the previous
        # qc's normalization/outproj tail are emitted as fillers inside
        # attention qc=n so the PE's exp-wait gaps get matmul work and the
        # scalar engine's exp stream never starves ----
        for piece in qkv_pieces(0):
            piece()
        prev_tail = []
        for n in range(4):
            fillers = (qkv_pieces(n + 1) if n + 1 < 4 else []) + prev_tail
            attn_qc(n, fillers)
            prev_tail = tail_pieces(n)
        for piece in prev_tail:
            piece()

        if _DEBUG:
            dbg_specs = [
                ("d_qt", qt, [128, S]),
                ("d_kt", kt, [128, S]),
                ("d_vT", vT, [128, S]),
                ("d_vx0", vx[0], [128, 131]),
                ("d_vx7", vx[7], [128, 131]),
                ("d_stageA", stageA, [65, S]),
                ("d_stageB", stageB, [65, S]),
                ("d_l2f", l2fA, [1, S]),
                ("d_l2rb", recAb, [1, S]),
                ("d_recBb", recBb, [1, S]),
                ("d_attnT", attnT, [128, S]),
            ]
            for nm, t, shp in dbg_specs:
                dt = F32 if t in (stageA, stageB, l2fA) else BF16
                d = nc.declare_dram_parameter(nm, shp, dt, isOutput=True)
                nc.sync.dma_start(d[:], t[:])

    nc.finalize()
    return nc


def _get_nc():
    if "nc" not in _CACHE:
        _CACHE["nc"] = _build()
    return _CACHE["nc"]


def kernel(hidden_states, cos, sin, w_qkv, w_o, _trace=False):
    hidden_states = np.asarray(hidden_states, dtype=np.float32)
    cos = np.asarray(cos, dtype=np.float32)
    sin = np.asarray(sin, dtype=np.float32)
    w_qkv = np.asarray(w_qkv, dtype=np.float32)
    w_o = np.asarray(w_o, dtype=np.float32)

    nc = _get_nc()

    # cos/sin tables in [d-row, position] layout, duplicated for 2 heads.
    # s2n is the sign-folded shifted sin: after the on-device partition
    # rotate-by-32 (u2[r] = u1[swap(r)]), u2 equals rotate_half(q) * sin.
    ct = cos.T  # [64, S]
    st = sin.T
    c2 = np.concatenate([ct, ct], axis=0).astype(BF16NP)
    s2 = np.concatenate([st, st], axis=0).astype(BF16NP)

    kl = np.arange(128)[:, None]
    ql = np.arange(128)[None, :]
    tri = (kl <= ql).astype(BF16NP)
    ident = np.eye(128).astype(BF16NP)

    in_maps = []
    for c in range(8):
        b, g = c // 4, c % 4
        heads = (2 * g, 2 * g + 1)
        wq = [w_qkv[:, h * 64:(h + 1) * 64] for h in heads]
        wk = [w_qkv[:, 512 + h * 64:512 + (h + 1) * 64] for h in heads]
        wv = [w_qkv[:, 1024 + h * 64:1024 + (h + 1) * 64] for h in heads]
        def _rot(w):
            return np.concatenate([-w[:, 32:], w[:, :32]], axis=1)

        wc = np.concatenate(
            [wq[0], wq[1], _rot(wq[0]), _rot(wq[1]),
             wk[0], wk[1], _rot(wk[0]), _rot(wk[1]),
             wv[0], wv[1]], axis=1)
        in_maps.append({
            "hidT": np.ascontiguousarray(
                hidden_states[b].T).astype(BF16NP),
            "wc": np.ascontiguousarray(wc).astype(BF16NP),
            "c2d": c2,
            "s2d": s2,
            "trid": tri,
            "identd": ident,
            "wod": np.ascontiguousarray(
                w_o[g * 128:(g + 1) * 128, :]).astype(BF16NP),
        })

    res = run_bass_kernel_spmd(nc, in_maps, list(range(8)), trace=_trace)
    _CACHE["last_result"] = res
    parts = [np.asarray(res.results[c]["out"], dtype=np.float32)
             for c in range(8)]
    full = np.stack([
        parts[0] + parts[1] + parts[2] + parts[3],
        parts[4] + parts[5] + parts[6] + parts[7],
    ])
    return full.astype(np.float32)


# revision 50
# speedup vs baseline: 1.0406x; 1.0320x over previous
"""Distributed Trainium2 kernel for causal multi-head attention with RoPE.

Problem: hidden[2,2048,512] -> qkv proj (8 heads x 64) -> RoPE -> causal
attention -> out proj [512,512] -> out [2,2048,512].

Sharding: 8 cores = (2 batches) x (4 head-pairs). Each core computes the
full attention pipeline for its batch and its 2 heads plus its slice of
the output projection (contraction over its 128 w_o rows); the host sums
the 4 partial outputs per batch.

v2 design notes (vs the v1 baseline at 157us):
- hidden pre-transposed + bf16 on host: no on-device transposes, half DMA.
- RoPE rotate-half folded into extra weight columns (x @ rot(w)); a DMA
  partition-shift variant measured the same (see kernel_v26_pending.py).
- scores: 2 heads row-packed in the PE array (K=64 each at tile rows 0/64)
  -> concurrent matmuls, 2x score throughput.
- exp on ScalarE in [128,1024] PSUM chunks (2 score blocks per ACTIVATE)
  to amortize the ~220cyc instruction overhead.
- causal mask multiply only on the 128-wide triangle slice of diagonal
  blocks (the rest of a diagonal block is unmasked).
- V in [k,d] layout via DMA-transpose; shared ones-columns give the
  softmax denominators as row 64 of the AV accumulators.
- normalization: l rows -> f32 stage tiles -> reciprocal_approx_fast ->
  ones-outer-product broadcast matmuls -> aligned tensor-tensor multiply.
- single summed [2048,512] bf16 output per core; host sums 4 per batch.
"""

import sys

import numpy as np

sys.path.insert(0, "/opt/trn_rl_repo")

import ml_dtypes  # noqa: E402

import concourse.bass as bass  # noqa: E402
import concourse.mybir as mybir  # noqa: E402
import concourse.tile as tile  # noqa: E402
from concourse import bacc  # noqa: E402
from concourse.bass_utils import run_bass_kernel_spmd  # noqa: E402

B, S, HID = 2, 2048, 512
F32 = mybir.dt.float32
BF16 = mybir.dt.bfloat16
BF16NP = ml_dtypes.bfloat16

_CACHE = {}
_DEBUG = False


def _build():
    nc = bacc.Bacc(None)

    hidT = nc.declare_dram_parameter("hidT", [HID, S], BF16, isOutput=False)
    wc = nc.declare_dram_parameter("wc", [HID, 640], BF16, isOutput=False)
    c2d = nc.declare_dram_parameter("c2d", [128, S], BF16, isOutput=False)
    s2d = nc.declare_dram_parameter("s2d", [128, S], BF16, isOutput=False)
    trid = nc.declare_dram_parameter("trid", [128, 128], BF16, isOutput=False)
    identd = nc.declare_dram_parameter("identd", [128, 128], BF16,
                                       isOutput=False)
    wod = nc.declare_dram_parameter("wod", [128, HID], BF16, isOutput=False)
    out = nc.declare_dram_parameter("out", [S, HID], BF16, isOutput=True)

    Exp = mybir.ActivationFunctionType.Exp

    with tile.TileContext(nc) as tc, \
         tc.tile_pool(name="const", bufs=1) as constp, \
         tc.tile_pool(name="big", bufs=1) as bigp, \
         tc.tile_pool(name="work", bufs=4) as workp, \
         tc.tile_pool(name="ps", bufs=2, space="PSUM") as psp:

        # ---- constants / inputs: hidden on the sync queue, weights and
        # tables on the scalar queue, so the first QKV matmul's operands
        # land as early as possible ----
        hT = []
        for kc in range(4):
            t = bigp.tile([128, S], BF16, name=f"hT{kc}")
            eng = nc.sync if kc % 2 == 0 else nc.gpsimd
            eng.dma_start(t[:], hidT[kc * 128:(kc + 1) * 128, :])
            hT.append(t)
        wcs = constp.tile([128, 4 * 640], BF16, name="wcs")
        for kc in range(4):
            nc.scalar.dma_start(wcs[:, kc * 640:(kc + 1) * 640],
                                wc[kc * 128:(kc + 1) * 128, :])
        c2 = constp.tile([128, S], BF16, name="c2")
        nc.scalar.dma_start(c2[:], c2d[:])
        s2 = constp.tile([128, S], BF16, name="s2")
        nc.scalar.dma_start(s2[:], s2d[:])
        tri = constp.tile([128, 128], BF16, name="tri")
        nc.scalar.dma_start(tri[:], trid[:])
        identb = constp.tile([128, 128], BF16, name="identb")
        nc.scalar.dma_start(identb[:], identd[:])
        wob = constp.tile([128, HID], BF16, name="wob")
        nc.scalar.dma_start(wob[:], wod[:])
        ones1 = constp.tile([1, 64], BF16, name="ones1")
        nc.vector.memset(ones1[:], 1.0)

        # PE warm-up: ~4us of dummy matmuls with no DMA dependencies so the
        # HAM clock-gate releases (1.2 -> 2.4 GHz) before the first real QKV
        # matmuls; they fill the otherwise-idle preamble/load window.
        warm = constp.tile([128, 512], BF16, name="warm")
        nc.vector.memset(warm[:], 0.0)
        for w in range(10):
            pswu = psp.tile([128, 1024], F32, name=f"wu{w}", tag="sc",
                            bufs=2)
            nc.tensor.matmul(pswu[:, 0:512], warm[:, 0:128], warm[:],
                             start=True, stop=True)

        # persistent activations
        qt = bigp.tile([128, S], BF16, name="qt")
        kt = bigp.tile([128, S], BF16, name="kt")
        vT = bigp.tile([128, S], BF16, name="vT")
        stageA = bigp.tile([65, S], F32, name="stageA")
        stageB = bigp.tile([65, S], F32, name="stageB")
        attnT = bigp.tile([128, S], BF16, name="attnT")
        attnB = bigp.tile([64, S], BF16, name="attnB")
        l2fA = bigp.tile([1, S], F32, name="l2fA")
        l2fB = bigp.tile([1, S], F32, name="l2fB")
        recAb = bigp.tile([1, S], BF16, name="recAb")
        recBb = bigp.tile([1, S], BF16, name="recBb")

        # v in [k, d] layout, 16 blocks of [128, 131]:
        # cols 0:64 = v_h0, 64 = ones, 65 unused, 66:130 = v_h1, 130 = ones
        vx = []
        for kb in range(16):
            t = bigp.tile([128, 131], BF16, name=f"vx{kb}")
            vx.append(t)

        def qkv_pieces(n):
            # wc col groups: [q2 | k2 | v2], 128 each. rotate_half comes
            # from an SBUF->SBUF DMA partition rotate-by-32 of u1 = q*s2n
            # (s2n is the sign-folded shifted sin table). Returns emission
            # closures so attention can interleave them into its exp-wait
            # gaps (keeps the PE warm, ACT never idles).
            nsl = slice(n * 512, (n + 1) * 512)
            pieces = []

            def rope(dst, lo):
                ps = psp.tile([128, 512], F32, name=f"ps{n}_{lo}",
                              tag="qkv", bufs=2)
                for kc in range(4):
                    nc.tensor.matmul(
                        ps[:],
                        wcs[:, kc * 640 + lo:kc * 640 + lo + 128],
                        hT[kc][:, nsl],
                        start=(kc == 0), stop=(kc == 3),
                    )
                psr = psp.tile([128, 512], F32, name=f"psr{n}_{lo}",
                               tag="qkv", bufs=2)
                for kc in range(4):
                    nc.tensor.matmul(
                        psr[:],
                        wcs[:, kc * 640 + lo + 128:kc * 640 + lo + 256],
                        hT[kc][:, nsl],
                        start=(kc == 0), stop=(kc == 3),
                    )
                t1 = workp.tile([128, 512], BF16, name="t1", tag="t1", bufs=2)
                nc.vector.tensor_mul(t1[:], ps[:], c2[:, nsl])
                t2 = workp.tile([128, 512], BF16, name="t2", tag="t2", bufs=2)
                nc.vector.tensor_mul(t2[:], psr[:], s2[:, nsl])
                nc.vector.tensor_add(dst[:, nsl], t1[:], t2[:])

            def vproj():
                ps = psp.tile([128, 512], F32, name=f"psv{n}",
                              tag="qkv", bufs=2)
                for kc in range(4):
                    nc.tensor.matmul(
                        ps[:],
                        wcs[:, kc * 640 + 512:kc * 640 + 640],
                        hT[kc][:, nsl],
                        start=(kc == 0), stop=(kc == 3),
                    )
                nc.vector.tensor_copy(vT[:, nsl], ps[:])

            def vxa():
                # transpose v chunk on the PE, then split columns on gpsimd
                tpv = psp.tile([128, 512], BF16, name=f"tpv{n}", tag="qkv",
                               bufs=2)
                for i in range(4):
                    kb = 4 * n + i
                    nc.tensor.transpose(tpv[:, i * 128:(i + 1) * 128],
                                        vT[:, kb * 128:(kb + 1) * 128],
                                        identb[:])
                vt4 = workp.tile([128, 512], BF16, name="vt4", tag="vt4",
                                 bufs=2)
                nc.vector.tensor_copy(vt4[:], tpv[:])
                for i in range(4):
                    kb = 4 * n + i
                    nc.gpsimd.tensor_copy(vx[kb][:, 0:64],
                                          vt4[:, i * 128:i * 128 + 64])
                    nc.gpsimd.tensor_copy(vx[kb][:, 66:130],
                                          vt4[:, i * 128 + 64:i * 128 + 128])
                    nc.vector.memset(vx[kb][:, 64:65], 1.0)
                    nc.vector.memset(vx[kb][:, 130:131], 1.0)

            pieces.append(lambda: rope(qt, 0))
            pieces.append(lambda: rope(kt, 256))
            pieces.append(vproj)
            pieces.append(vxa)
            return pieces

        def attn_qc(qc, fillers=(), prelude=()):
            fillers = list(fillers)
            nkb = 4 * qc + 4
            qbase = qc * 512
            accs = []
            for h in range(2):
                acc = psp.tile([128, 512], F32, name=f"acc{h}_{qc}",
                               tag="acc", bufs=2)
                accs.append(acc)
            # chunks of 2 kb blocks; diag block j (= kb - 4*qc >= 0) is
            # trimmed to live q cols [128j, 512)
            chunks = []
            for kb0 in range(0, nkb, 2):
                blocks = []  # (kb, q0b, psum col offset, len)
                off = 0
                for kb in range(kb0, min(kb0 + 2, nkb)):
                    j = kb - 4 * qc
                    q0b = max(0, 128 * j)
                    ln = 512 - q0b
                    blocks.append((kb, q0b, off, ln))
                    off += ln
                chunks.append((blocks, off))

            scs = {}

            def mm_chunk(i, h):
                blocks, off = chunks[i]
                hsl = slice(h * 64, (h + 1) * 64)
                sc = psp.tile([128, 1024], F32, name=f"sc{h}",
                              tag="sc", bufs=2)
                for kb, q0b, o, ln in blocks:
                    nc.tensor.matmul(
                        sc[:, o:o + ln],
                        kt[hsl, kb * 128:(kb + 1) * 128],
                        qt[hsl, qbase + q0b:qbase + 512],
                        start=True, stop=True,
                    )
                scs[(i, h)] = sc

            # software pipeline: exp/AV of chunk i interleave with the
            # score matmuls of chunk i+1 (per head, matching the sc ring)
            mm_chunk(0, 0)
            mm_chunk(0, 1)
            for p in prelude:
                p()  # emitted before any AV: AV inputs (vx) may come here
            for i, (blocks, off) in enumerate(chunks):
                for h in range(2):
                    pr = workp.tile([128, 1024], BF16, name=f"pr{h}",
                                    tag="probs", bufs=6)
                    nc.scalar.activation(pr[:, 0:off], scs.pop((i, h))[:, 0:off],
                                         Exp, scale=0.125)
                    lsl = slice(h * 65 + h, h * 65 + h + 65)  # 0:65 / 66:131
                    for kb, q0b, o, ln in blocks:
                        j = kb - 4 * qc
                        if j >= 0:
                            # mask the 128-wide triangle slice in place;
                            # the rest of a diagonal block is unmasked
                            nc.vector.tensor_mul(pr[:, o:o + 128],
                                                 pr[:, o:o + 128],
                                                 tri[:])
                        nc.tensor.matmul(
                            accs[h][0:65, q0b:512],
                            vx[kb][:, lsl],
                            pr[:, o:o + ln],
                            start=(kb == 0), stop=(kb == nkb - 1),
                        )
                    if i + 1 < len(chunks):
                        mm_chunk(i + 1, h)
                    if fillers:
                        fillers.pop(0)()
            for f in fillers:
                f()
            qsl = slice(qbase, qbase + 512)
            nc.vector.tensor_copy(stageA[:, qsl], accs[0][0:65, :])
            nc.vector.tensor_copy(stageB[:, qsl], accs[1][0:65, :])
            # denominators to base-0 f32 rows (partition shift via DMA)
            nc.sync.dma_start(l2fA[0:1, qsl], stageA[64:65, qsl])
            nc.sync.dma_start(l2fB[0:1, qsl], stageB[64:65, qsl])

        def tail_pieces(qc):
            # recip + normalize + outproj for one qc, as filler closures
            # emitted inside the NEXT qc's attention loop
            qsl = slice(qc * 512, (qc + 1) * 512)

            def recip():
                for lf, rb in ((l2fA, recAb), (l2fB, recBb)):
                    rec = workp.tile([1, 512], F32, name="rec", tag="rec",
                                     bufs=2)
                    nc.vector.reciprocal_approx_fast(rec[:], lf[0:1, qsl])
                    nc.vector.tensor_copy(rb[0:1, qsl], rec[:])

            def norm(h):
                recb = psp.tile([64, 512], F32, name=f"recb{h}",
                                tag="qkv", bufs=2)
                src = recAb if h == 0 else recBb
                nc.tensor.matmul(recb[:], ones1[:], src[0:1, qsl],
                                 start=True, stop=True)
                if h == 0:
                    nc.vector.tensor_mul(attnT[0:64, qsl],
                                         stageA[0:64, qsl], recb[:])
                else:
                    nc.vector.tensor_mul(attnB[0:64, qsl],
                                         stageB[0:64, qsl], recb[:])
                    nc.sync.dma_start(attnT[64:128, qsl], attnB[0:64, qsl])

            def outproj(mc):
                oP = psp.tile([128, 512], F32, name=f"oP{mc}",
                              tag="qkv", bufs=2)
                nc.tensor.matmul(oP[:], attnT[:, mc * 128:(mc + 1) * 128],
                                 wob[:], start=True, stop=True)
                osb = workp.tile([128, 512], BF16, name="osb", tag="osb",
                                 bufs=3)
                nc.vector.tensor_copy(osb[:], oP[:])
                eng = nc.gpsimd if mc % 2 == 0 else nc.sync
                eng.dma_start(out[mc * 128:(mc + 1) * 128, :], osb[:])

            pieces = [recip, lambda: norm(0), lambda: norm(1)]
            pieces.extend(
                (lambda m: lambda: outproj(m))(mc)
                for mc in range(4 * qc, 4 * qc + 4))
            return pieces

        # ---- pipelined main loop: qkv/vx for chunk n+1 AND the previous
        # qc's normalization/outproj tail are emitted as fillers inside
        # attention qc=n so the PE's exp-wait gaps get matmul work and the
        # scalar engine's exp stream never starves ----
        for piece in qkv_pieces(0):
            piece()
        prev_tail = []
        for n in range(4):
            fillers = (qkv_pieces(n + 1) if n + 1 < 4 else []) + prev_tail
            attn_qc(n, fillers)
            prev_tail = tail_pieces(n)
        for piece in prev_tail:
            piece()

        if _DEBUG:
            dbg_specs = [
                ("d_qt", qt, [128, S]),
                ("d_kt", kt, [128, S]),
                ("d_vT", vT, [128, S]),
                ("d_vx0", vx[0], [128, 131]),
                ("d_vx7", vx[7], [128, 131]),
                ("d_stageA", stageA, [65, S]),
                ("d_stageB", stageB, [65, S]),
                ("d_l2f", l2fA, [1, S]),
                ("d_l2rb", recAb, [1, S]),
                ("d_recBb", recBb, [1, S]),
                ("d_attnT", attnT, [128, S]),
            ]
            for nm, t, shp in dbg_specs:
                dt = F32 if t in (stageA, stageB, l2fA) else BF16
                d = nc.declare_dram_parameter(nm, shp, dt, isOutput=True)
                nc.sync.dma_start(d[:], t[:])

    nc.finalize()
    return nc


def _get_nc():
    if "nc" not in _CACHE:
        _CACHE["nc"] = _build()
    return _CACHE["nc"]


def kernel(hidden_states, cos, sin, w_qkv, w_o, _trace=False):
    hidden_states = np.asarray(hidden_states, dtype=np.float32)
    cos = np.asarray(cos, dtype=np.float32)
    sin = np.asarray(sin, dtype=np.float32)
    w_qkv = np.asarray(w_qkv, dtype=np.float32)
    w_o = np.asarray(w_o, dtype=np.float32)

    nc = _get_nc()

    # cos/sin tables in [d-row, position] layout, duplicated for 2 heads.
    # s2n is the sign-folded shifted sin: after the on-device partition
    # rotate-by-32 (u2[r] = u1[swap(r)]), u2 equals rotate_half(q) * sin.
    ct = cos.T  # [64, S]
    st = sin.T
    c2 = np.concatenate([ct, ct], axis=0).astype(BF16NP)
    s2 = np.concatenate([st, st], axis=0).astype(BF16NP)

    kl = np.arange(128)[:, None]
    ql = np.arange(128)[None, :]
    tri = (kl <= ql).astype(BF16NP)
    ident = np.eye(128).astype(BF16NP)

    in_maps = []
    for c in range(8):
        b, g = c // 4, c % 4
        heads = (2 * g, 2 * g + 1)
        wq = [w_qkv[:, h * 64:(h + 1) * 64] for h in heads]
        wk = [w_qkv[:, 512 + h * 64:512 + (h + 1) * 64] for h in heads]
        wv = [w_qkv[:, 1024 + h * 64:1024 + (h + 1) * 64] for h in heads]
        def _rot(w):
            return np.concatenate([-w[:, 32:], w[:, :32]], axis=1)

        wc = np.concatenate(
            [wq[0], wq[1], _rot(wq[0]), _rot(wq[1]),
             wk[0], wk[1], _rot(wk[0]), _rot(wk[1]),
             wv[0], wv[1]], axis=1)
        in_maps.append({
            "hidT": np.ascontiguousarray(
                hidden_states[b].T).astype(BF16NP),
            "wc": np.ascontiguousarray(wc).astype(BF16NP),
            "c2d": c2,
            "s2d": s2,
            "trid": tri,
            "identd": ident,
            "wod": np.ascontiguousarray(
                w_o[g * 128:(g + 1) * 128, :]).astype(BF16NP),
        })

    res = run_bass_kernel_spmd(nc, in_maps, list(range(8)), trace=_trace)
    _CACHE["last_result"] = res
    parts = [np.asarray(res.results[c]["out"], dtype=np.float32)
             for c in range(8)]
    full = np.stack([
        parts[0] + parts[1] + parts[2] + parts[3],
        parts[4] + parts[5] + parts[6] + parts[7],
    ])
    return full.astype(np.float32)
